# revision 7
# baseline (speedup 1.0000x reference)
"""SupCon contrastive-loss kernel for 8 Trainium2 NeuronCores.

Row-sharded: core c computes rows [c*1024, (c+1)*1024) of the 8192x8192
logits / perfect_logit matrices plus per-row sum(exp(logits)) partials.
The scalar loss is assembled on host in f64 from O(N) ingredients:
  sum_j pos_mask*logits  == fn_i . g[lab_i] * 10 - diag_i   (class-sum trick)
  denom_i                == sall_i - exp(diag_i)
Device does all O(N^2) and O(N*C) compute; host does O(N) assembly.
"""

import numpy as np

import concourse.bass as bass
from concourse import mybir
from concourse.bacc import Bacc
from concourse.tile import TileContext
from concourse.bass_utils import run_bass_kernel_spmd

N = 8192
C = 256
NCORES = 8
R = N // NCORES          # 1024 rows per core
RT = R // 128            # 8 row tiles per core
CHUNK = 2048             # columns processed per psum group
NCHUNK = N // CHUNK      # 4
TEMP = 0.1
INV_TEMP = 1.0 / TEMP    # 10.0
NUM_CLASSES = 1000

F32 = mybir.dt.float32
BF16 = mybir.dt.bfloat16
AF = mybir.ActivationFunctionType
ALU = mybir.AluOpType

# matmul dtype for the big NxN product: float32 is exact (1/4 PE rate),
# float32r is full rate but reduced precision. Chosen empirically.
MM_DT = mybir.dt.float32

_PROGRAM_CACHE = {}


def _bcast_ap(ap, parts):
    """Replicate a [1, X]-ish AP across `parts` partitions (step-0 partition dim)."""
    return bass.AP(tensor=ap.tensor, offset=ap.offset, ap=[[0, parts], *ap.ap])


def _build_program():
    if "nc" in _PROGRAM_CACHE:
        return _PROGRAM_CACHE["nc"]

    nc = Bacc()

    ft_d = nc.declare_dram_parameter("ft", [2, 128, N], F32, isOutput=False)
    ftr_d = nc.declare_dram_parameter("ftr", [2, 128, R], F32, isOutput=False)
    lab_d = nc.declare_dram_parameter("lab", [N], F32, isOutput=False)
    labr_d = nc.declare_dram_parameter("labr", [128, RT], F32, isOutput=False)
    logits_d = nc.declare_dram_parameter("logits_out", [RT, 128, N], F32, isOutput=True)
    perf_d = nc.declare_dram_parameter("perf_out", [RT, 128, N], BF16, isOutput=True)
    sall_d = nc.declare_dram_parameter("sall_out", [RT, 128, NCHUNK], F32, isOutput=True)

    with TileContext(nc) as tc:
        with (
            tc.tile_pool(name="const", bufs=1) as const,
            tc.tile_pool(name="norm", bufs=1) as norm,
            tc.tile_pool(name="small", bufs=3) as small,
            tc.tile_pool(name="lgp", bufs=3) as lgp,
            tc.tile_pool(name="pfp", bufs=3) as pfp,
            tc.tile_pool(name="exp_pool", bufs=2) as exp_pool,
            tc.tile_pool(name="sallp", bufs=2) as sallp,
            tc.tile_pool(name="psum", bufs=2, space="PSUM") as psum,
        ):
            # ---- load inputs ----
            ft0 = const.tile([128, N], F32, name="ft0")
            ft1 = const.tile([128, N], F32, name="ft1")
            nc.sync.dma_start(out=ft0, in_=ft_d[0])
            nc.sync.dma_start(out=ft1, in_=ft_d[1])
            ftr0 = const.tile([128, R], F32, name="ftr0")
            ftr1 = const.tile([128, R], F32, name="ftr1")
            nc.sync.dma_start(out=ftr0, in_=ftr_d[0])
            nc.sync.dma_start(out=ftr1, in_=ftr_d[1])

            lab_bc = const.tile([128, N], F32, name="lab_bc")
            nc.gpsimd.dma_start(out=lab_bc, in_=_bcast_ap(lab_d[:], 128))
            labr = const.tile([128, RT], F32, name="labr")
            nc.sync.dma_start(out=labr, in_=labr_d[:])

            ones = const.tile([128, 128], F32, name="ones")
            nc.vector.memset(ones, 1.0)

            # ---- rnorm = 1/sqrt(colsum(ft^2)) broadcast across partitions ----
            # colsum via ones-matmul (output replicated on all 128 partitions),
            # then rnorm = exp(-0.5 * ln(normsq)) on ACT (Rsqrt is banned).
            rn = norm.tile([128, N], F32, name="rn")
            rnr = norm.tile([128, R], F32, name="rnr")
            for dst, srcs, width in ((rn, (ft0, ft1), N), (rnr, (ftr0, ftr1), R)):
                for t in range(width // 512):
                    s = slice(t * 512, (t + 1) * 512)
                    ps = psum.tile([128, CHUNK], F32, tag="ps", name="ps_norm")
                    for ci, src in enumerate(srcs):
                        sq = small.tile([128, 512], F32, tag="sq", name="sq")
                        nc.vector.tensor_mul(sq, src[:, s], src[:, s])
                        nc.tensor.matmul(
                            ps[:, :512], lhsT=ones, rhs=sq,
                            start=(ci == 0), stop=(ci == 1),
                        )
                    lg = small.tile([128, 512], F32, tag="lgn", name="lgn")
                    nc.scalar.activation(lg, ps[:, :512], AF.Ln)
                    nc.scalar.activation(dst[:, s], lg, AF.Exp, scale=-0.5)

            # ---- normalize in place: fnT = ftT * rnorm ----
            for q in range(4):
                s = slice(q * (N // 4), (q + 1) * (N // 4))
                nc.vector.tensor_mul(ft0[:, s], ft0[:, s], rn[:, s])
                nc.vector.tensor_mul(ft1[:, s], ft1[:, s], rn[:, s])
            nc.vector.tensor_mul(ftr0, ftr0, rnr)
            nc.vector.tensor_mul(ftr1, ftr1, rnr)

            if MM_DT != F32:
                ft0 = ft0.bitcast(MM_DT)
                ft1 = ft1.bitcast(MM_DT)
                ftr0 = ftr0.bitcast(MM_DT)
                ftr1 = ftr1.bitcast(MM_DT)

            # ---- main loop: 8 row tiles x 4 column chunks ----
            for r in range(RT):
                rs = slice(r * 128, (r + 1) * 128)
                sall = sallp.tile([128, NCHUNK], F32, tag="sall", name="sall")
                for ch in range(NCHUNK):
                    off = ch * CHUNK
                    ps = psum.tile([128, CHUNK], F32, tag="ps", name="ps_main")
                    for ci, (ftk, ftrk) in enumerate(((ft0, ftr0), (ft1, ftr1))):
                        for t in range(CHUNK // 512):
                            s = slice(off + t * 512, off + (t + 1) * 512)
                            nc.tensor.matmul(
                                ps[:, t * 512:(t + 1) * 512],
                                lhsT=ftrk[:, rs], rhs=ftk[:, s],
                                start=(ci == 0), stop=(ci == 1),
                            )
                    # logits = psum * 10  (ACT copy with scale)
                    lg = lgp.tile([128, CHUNK], F32, tag="lg", name="lg")
                    nc.scalar.activation(lg, ps, AF.Copy, scale=INV_TEMP)
                    # exp(10*psum) + row-sum accumulator
                    ex = exp_pool.tile([128, CHUNK], F32, tag="ex", name="ex")
                    nc.scalar.activation(
                        ex, ps, AF.Exp, scale=INV_TEMP,
                        accum_out=sall[:, ch:ch + 1],
                    )
                    # perfect/2 = (lab == lab_i) - 0.5  -> bf16 (+-0.5 exact);
                    # host multiplies by 2.
                    pf = pfp.tile([128, CHUNK], BF16, tag="pf", name="pf")
                    nc.vector.tensor_scalar(
                        out=pf, in0=lab_bc[:, off:off + CHUNK],
                        scalar1=labr[:, r:r + 1], scalar2=0.5,
                        op0=ALU.is_equal, op1=ALU.subtract,
                    )
                    nc.sync.dma_start(out=logits_d[r, :, off:off + CHUNK], in_=lg)
                    nc.sync.dma_start(out=perf_d[r, :, off:off + CHUNK], in_=pf)
                nc.sync.dma_start(out=sall_d[r], in_=sall)

    nc.finalize()
    _PROGRAM_CACHE["nc"] = nc
    return nc


def _make_in_maps(f, labf):
    ft = np.ascontiguousarray(f.T).reshape(2, 128, N)
    in_maps = []
    for c in range(NCORES):
        rows = slice(c * R, (c + 1) * R)
        ftr = np.ascontiguousarray(f[rows].T).reshape(2, 128, R)
        labr = np.ascontiguousarray(labf[rows].reshape(RT, 128).T)
        in_maps.append({"ft": ft, "ftr": ftr, "lab": labf, "labr": labr})
    return in_maps


def _assemble(results, f, labels_int):
    logits = np.concatenate(
        [np.asarray(r["logits_out"]).reshape(R, N) for r in results], axis=0
    )
    perfect = np.concatenate(
        [np.asarray(r["perf_out"]).astype(np.float32).reshape(R, N) for r in results],
        axis=0,
    ) * np.float32(2.0)
    sall = np.concatenate(
        [np.asarray(r["sall_out"]).astype(np.float64).sum(-1).reshape(R) for r in results],
        axis=0,
    )

    # host-side O(N) loss assembly in f64
    fn = f.astype(np.float64)
    fn /= np.maximum(np.linalg.norm(fn, axis=1, keepdims=True), 1e-8)
    g = np.zeros((NUM_CLASSES, C), np.float64)
    np.add.at(g, labels_int, fn)
    d = np.diag(logits).astype(np.float64)
    # sum over same-class j (incl. diagonal) of logits[i, j], minus diagonal
    p_pos = (fn * g[labels_int]).sum(axis=1) * INV_TEMP - d
    cnt = np.bincount(labels_int, minlength=NUM_CLASSES)[labels_int].astype(np.float64) - 1.0
    denom = sall - np.exp(d)
    mean_log_prob_pos = (p_pos - cnt * np.log(denom)) / (cnt + 1e-6)
    loss = np.float32(-(TEMP / TEMP) * mean_log_prob_pos.mean())
    return loss, logits, perfect


def _run(features, labels, trace=False):
    f = np.asarray(features, np.float32).reshape(N, C)
    labels_int = np.asarray(labels).reshape(N).astype(np.int64)
    labf = labels_int.astype(np.float32)

    nc = _build_program()
    res = run_bass_kernel_spmd(
        nc, _make_in_maps(f, labf), list(range(NCORES)), trace=trace
    )
    out = _assemble(res.results, f, labels_int)
    return out, res


def kernel(features, labels):
    out, _ = _run(features, labels, trace=False)
    return out
